# revision 6
# baseline (speedup 1.0000x reference)
"""Trainium2 Bass kernel for nn_DenseRelativeLoc.

Data-parallel over batch: 32 images per NeuronCore x 8 cores.

v2 design (vs bf16 baseline):
  * Projection z = x^T @ W1 runs in fp8 (e4m3) DoubleRow matmuls with a
    two-term error compensation:
        z*256 = x_hi @ (256*W_hi) + x_lo @ (16*W_hi) + x_hi @ (256*W_lo)
    where x_hi = fp8(x), x_lo = fp8(16*(x - x_hi)), W_hi = fp8(16*W1)/16,
    W_lo ~ fp8(256*(W1 - W_hi)). Residual error ~2^-8 (bf16-level), at
    0.75x the bf16 cycle cost (9 DoubleRow K=256 matmuls vs 6 bf16 K=128
    per 768-deep contraction... per j-chunk of 128 output channels).
  * W1-stationary orientation: out zT[hid_chunk 128, 392 cols] per
    (pair-of-batches, branch, j) -- no partition waste (196 = 128+68
    chunking eliminated).
  * The sample gather runs on the GPSIMD engine via ap_gather (free-dim
    gather of [hw, j] blocks), replacing one-hot matmuls + is_equal
    one-hot construction entirely. Index tables precomputed on host.
  * GEMM2 / GEMM3 stay bf16 on the PE. h1 = relu(ga + gb + b1) built by
    DVE adds + Scalar relu.
Pipeline is pair-granular, tail (gemm2/3) lagged by 2 pairs.
"""
import sys
import types

import numpy as np

B, C, H, W_IMG = 256, 768, 14, 14
S = 196          # sample count == H*W
HID = 512
OUT = 2
N_CORES = 8
BPC = B // N_CORES      # batches per core
PAIRS = BPC // 2        # 16
KC = C // 128           # 6 contraction chunks per branch
MJ = HID // 128         # 4 HID chunks
S2 = 2 * S              # 392: pair width
PAIRW = 400             # padded pair width in x2 (16-aligned)
SPAD = 208              # per-col padded sample count for gather idx
NIDX = 2 * SPAD         # 416 gather indices per (pair, branch)
IDXW = NIDX // 16       # 26 idx words per partition
XGP = 4                 # pairs per x-load group
NXG = PAIRS // XGP      # 4 groups


def _install_ntff_hook():
    try:
        import antenv.axon_hooks  # noqa: F401
        return
    except ImportError:
        pass
    try:
        from trn_agent_boot.trn_boot import _ntff_profile_via_ctypes
        hook = _ntff_profile_via_ctypes("/opt/axon/libaxon_pjrt.so")
    except Exception:
        hook = None
    mod = types.ModuleType("antenv.axon_hooks")
    mod.get_axon_ntff_profile_hook = lambda: hook
    sys.modules["antenv.axon_hooks"] = mod


def _build_nc():
    from contextlib import ExitStack

    import concourse.bass as bass
    import concourse.bacc as bacc
    import concourse.mybir as mybir
    import concourse.tile as tile

    dt = mybir.dt
    f32, bf16, i32, i16 = dt.float32, dt.bfloat16, dt.int32, dt.int16
    f8 = dt.float8e4
    AF = mybir.ActivationFunctionType
    ALU = mybir.AluOpType
    DR = mybir.MatmulPerfMode.DoubleRow

    nc = bacc.Bacc(None, target_bir_lowering=False)

    # x2: [128, 2*KC (hv*KC+k), PAIRS*PAIRW] fp8; pair p cols [p*400, p*400+392)
    x2_t = nc.dram_tensor("x2", [128, 2 * KC, PAIRS * PAIRW], f8,
                          kind="ExternalInput")
    w1h256_t = nc.dram_tensor("W1h256", [128, 2 * KC, HID], f8,
                              kind="ExternalInput")
    w1h16_t = nc.dram_tensor("W1h16", [128, 2 * KC, HID], f8,
                             kind="ExternalInput")
    w1lo_t = nc.dram_tensor("W1lo", [128, 2 * KC, HID], f8,
                            kind="ExternalInput")
    w2_t = nc.dram_tensor("W2", [HID, HID], bf16, kind="ExternalInput")
    w3_t = nc.dram_tensor("W3", [HID, OUT], bf16, kind="ExternalInput")
    b1_t = nc.dram_tensor("b1", [HID], f32, kind="ExternalInput")
    b2_t = nc.dram_tensor("b2", [HID], f32, kind="ExternalInput")
    b3_t = nc.dram_tensor("b3", [OUT], f32, kind="ExternalInput")
    idx_t = nc.dram_tensor("idxg", [128, PAIRS * 2 * IDXW], i16,
                           kind="ExternalInput")
    pxs_t = nc.dram_tensor("pxs", [BPC, S2], i32, kind="ExternalInput")
    pys_t = nc.dram_tensor("pys", [BPC, S2], i32, kind="ExternalInput")
    pred_t = nc.dram_tensor("predT", [OUT, BPC * S], f32, kind="ExternalOutput")
    delta_t = nc.dram_tensor("deltaxy", [BPC * S, OUT], f32,
                             kind="ExternalOutput")

    with ExitStack() as ctx:
        tc = ctx.enter_context(tile.TileContext(nc))
        wpool = ctx.enter_context(tc.tile_pool(name="w", bufs=1))
        xpool = ctx.enter_context(tc.tile_pool(name="xg", bufs=3))
        zspool = ctx.enter_context(tc.tile_pool(name="zs", bufs=2))
        gpool = ctx.enter_context(tc.tile_pool(name="g", bufs=2))
        tpool = ctx.enter_context(tc.tile_pool(name="tmp", bufs=2))
        h1pool = ctx.enter_context(tc.tile_pool(name="h1", bufs=2))
        h2pool = ctx.enter_context(tc.tile_pool(name="h2", bufs=2))
        opool = ctx.enter_context(tc.tile_pool(name="op", bufs=1))
        idxpool = ctx.enter_context(tc.tile_pool(name="idx", bufs=1))
        zps = ctx.enter_context(tc.tile_pool(name="zps", bufs=1, space="PSUM"))
        hps = ctx.enter_context(tc.tile_pool(name="hps", bufs=1, space="PSUM"))

        # ---------- x prefetch groups (XGP pairs per group) ----------
        xg_tiles = {}

        def emit_xgroup(g):
            if g in xg_tiles or g >= NXG:
                return
            xt = xpool.tile([128, 2 * KC, XGP * PAIRW], f8, name=f"xg{g}",
                            tag="xg")
            nc.sync.dma_start(
                xt[:],
                bass.AP(x2_t, g * XGP * PAIRW,
                        [[2 * KC * PAIRS * PAIRW, 128],
                         [PAIRS * PAIRW, 2 * KC],
                         [1, XGP * PAIRW]]),
            )
            xg_tiles[g] = xt

        emit_xgroup(0)
        emit_xgroup(1)

        # ---------- small tensors ----------
        idxt = idxpool.tile([128, PAIRS * 2 * IDXW], i16, name="idxt",
                            tag="idxt")
        nc.sync.dma_start(idxt[:], idx_t[:, :])

        pxs_sb = idxpool.tile([BPC, S2], i32, name="pxs_sb", tag="pxs_sb")
        nc.sync.dma_start(pxs_sb[:], pxs_t[:, :])
        pys_sb = idxpool.tile([BPC, S2], i32, name="pys_sb", tag="pys_sb")
        nc.sync.dma_start(pys_sb[:], pys_t[:, :])

        ones_row = wpool.tile([1, 128], bf16, name="ones_row", tag="ones_row")
        nc.vector.memset(ones_row[:], 1.0)

        # PE warm-up so the p-state ramps before real matmuls
        wmt = hps.tile([128, 128], f32, name="warm", tag="warm")
        for _ in range(24):
            nc.tensor.matmul(wmt[:], ones_row[:], ones_row[:],
                             start=True, stop=True)

        # ---------- weights ----------
        w1h256 = wpool.tile([128, 2 * KC, HID], f8, name="w1h256", tag="w1h256")
        nc.sync.dma_start(w1h256[:], w1h256_t[:, :, :])
        w1h16 = wpool.tile([128, 2 * KC, HID], f8, name="w1h16", tag="w1h16")
        nc.sync.dma_start(w1h16[:], w1h16_t[:, :, :])
        w1lo = wpool.tile([128, 2 * KC, HID], f8, name="w1lo", tag="w1lo")
        nc.sync.dma_start(w1lo[:], w1lo_t[:, :, :])

        w2b = []
        for k in range(MJ):
            wb = wpool.tile([128, HID], bf16, name=f"w2b{k}", tag=f"w2b{k}")
            nc.sync.dma_start(wb[:], w2_t[k * 128:(k + 1) * 128, :])
            w2b.append(wb)
        w3b = []
        for k in range(MJ):
            wb = wpool.tile([128, OUT], bf16, name=f"w3b{k}", tag=f"w3b{k}")
            nc.sync.dma_start(wb[:], w3_t[k * 128:(k + 1) * 128, :])
            w3b.append(wb)
        b1c, b2c = [], []
        for j in range(MJ):
            t1 = wpool.tile([128, 1], f32, name=f"b1c{j}", tag=f"b1c{j}")
            nc.sync.dma_start(t1[:], b1_t[j * 128:(j + 1) * 128])
            b1c.append(t1)
            t2 = wpool.tile([128, 1], f32, name=f"b2c{j}", tag=f"b2c{j}")
            nc.sync.dma_start(t2[:], b2_t[j * 128:(j + 1) * 128])
            b2c.append(t2)
        b3c = wpool.tile([OUT, 1], f32, name="b3c", tag="b3c")
        nc.sync.dma_start(b3c[:], b3_t[:])

        # ---------- deltaxy on gpsimd ----------
        dsub = idxpool.tile([BPC, S2], i32, name="dsub", tag="dsub")
        nc.gpsimd.tensor_tensor(dsub[:], pxs_sb[:], pys_sb[:], ALU.subtract)
        ddel = idxpool.tile([BPC, S2], f32, name="ddel", tag="ddel")
        nc.gpsimd.tensor_scalar(ddel[:], dsub[:], float(H - 1), None,
                                op0=ALU.add)
        nc.sync.dma_start(bass.AP(delta_t, 0, [[S2, BPC], [1, S2]]), ddel[:])

        pred_all = opool.tile([OUT, BPC * S], f32, name="pred_all",
                              tag="pred_all")

        # ---------- per-pair stages ----------
        zs_pair = {}     # P -> (zs_a, zs_b) [128, S2, MJ] bf16
        h1_pair = {}     # P -> [128, MJ, S2] bf16

        def emit_proj(P, br):
            """9 DoubleRow matmuls per j into zT psum, then scaled copy."""
            g, gp = divmod(P, XGP)
            xt = xg_tiles[g]
            if br == 0:
                za = zspool.tile([128, S2, MJ], bf16, name=f"zsa{P}", tag="zsa")
                zb = zspool.tile([128, S2, MJ], bf16, name=f"zsb{P}", tag="zsb")
                zs_pair[P] = (za, zb)
            zs = zs_pair[P][br]
            col0 = gp * PAIRW
            terms = ((w1h256, 0), (w1h16, 1), (w1lo, 0))
            for j in range(MJ):
                zt = zps.tile([128, S2], f32, name=f"zt{j}_{P}_{br}",
                              tag=f"zt{j}")
                n = 0
                for wt, hv in terms:
                    for t3 in range(KC // 2):
                        nc.tensor.matmul(
                            zt[:],
                            wt[:, br * KC + 2 * t3: br * KC + 2 * t3 + 2,
                               j * 128:(j + 1) * 128],
                            xt[:, hv * KC + 2 * t3:hv * KC + 2 * t3 + 2,
                               col0:col0 + S2],
                            start=(n == 0), stop=(n == 8),
                            perf_mode=DR,
                        )
                        n += 1
                dst = zs[:, :, j]
                if j < 2:
                    nc.scalar.activation(dst, zt[:], AF.Copy, scale=1 / 256.0)
                else:
                    nc.vector.tensor_scalar(dst, zt[:], 1 / 256.0, None,
                                            op0=ALU.mult)

        def emit_gather_h1(P):
            """ap_gather both branches on gpsimd, then h1 = relu(ga+gb+b1)."""
            za, zb = zs_pair.pop(P)
            ga = gpool.tile([128, NIDX, MJ], bf16, name=f"ga{P}", tag="ga")
            gb = gpool.tile([128, NIDX, MJ], bf16, name=f"gb{P}", tag="gb")
            for g_out, zs, br in ((ga, za, 0), (gb, zb, 1)):
                nc.gpsimd.ap_gather(
                    g_out[:], zs[:],
                    idxt[:, (P * 2 + br) * IDXW:(P * 2 + br + 1) * IDXW],
                    channels=128, num_elems=S2, d=MJ, num_idxs=NIDX,
                )
            h1 = h1pool.tile([128, MJ, S2], bf16, name=f"h1_{P}", tag="h1")
            h1_pair[P] = h1
            for j in range(MJ):
                for col in range(2):
                    tm = tpool.tile([128, S], bf16, name=f"tm{j}{col}_{P}",
                                    tag=f"tm{(2 * j + col) % 4}")
                    nc.vector.tensor_tensor(
                        tm[:],
                        ga[:, col * SPAD:col * SPAD + S, j],
                        gb[:, col * SPAD:col * SPAD + S, j],
                        ALU.add,
                    )
                    nc.scalar.activation(
                        h1[:, j, col * S:(col + 1) * S], tm[:], AF.Relu,
                        bias=b1c[j][:],
                    )

        def emit_tail(P):
            """GEMM2 + relu, GEMM3 + bias into pred_all."""
            h1 = h1_pair.pop(P)
            h2 = h2pool.tile([128, MJ, S2], bf16, name=f"h2_{P}", tag="h2")
            for j in range(MJ):
                hp = hps.tile([128, S2], f32, name=f"h2ps{j}_{P}",
                              tag=f"hps{j % 2}")
                for k in range(MJ):
                    nc.tensor.matmul(
                        hp[:],
                        w2b[k][:, j * 128:(j + 1) * 128],
                        h1[:, k, :],
                        start=(k == 0), stop=(k == MJ - 1),
                    )
                if j < 2:
                    nc.scalar.activation(h2[:, j, :], hp[:], AF.Relu,
                                         bias=b2c[j][:])
                else:
                    nc.vector.tensor_scalar(h2[:, j, :], hp[:], b2c[j][:],
                                            0.0, op0=ALU.add, op1=ALU.max)
            pp = hps.tile([OUT, S2], f32, name=f"pps_{P}", tag="pps")
            for k in range(MJ):
                nc.tensor.matmul(pp[:], w3b[k][:], h2[:, k, :],
                                 start=(k == 0), stop=(k == MJ - 1))
            nc.vector.tensor_scalar(
                pred_all[:, P * S2:(P + 1) * S2], pp[:], b3c[:], None,
                op0=ALU.add,
            )
            if P % 4 == 3:
                q = P // 4
                nc.sync.dma_start(
                    pred_t[:, q * 4 * S2:(q + 1) * 4 * S2],
                    pred_all[:, q * 4 * S2:(q + 1) * 4 * S2],
                )

        # ---------- main loop: tail lags proj by 2 pairs ----------
        for P in range(PAIRS):
            if P % XGP == 0:
                emit_xgroup(P // XGP + 1)
                emit_xgroup(P // XGP + 2)
            emit_proj(P, 0)
            emit_proj(P, 1)
            if P >= 2:
                emit_tail(P - 2)
            emit_gather_h1(P)
        emit_tail(PAIRS - 2)
        emit_tail(PAIRS - 1)

    nc.finalize()
    return nc


_NC = None


def _get_nc():
    global _NC
    if _NC is None:
        _install_ntff_hook()
        _NC = _build_nc()
    return _NC


def _make_in_maps(inputs):
    import ml_dtypes
    f8 = ml_dtypes.float8_e4m3
    bf16 = ml_dtypes.bfloat16

    x = np.asarray(inputs["x"], dtype=np.float32).reshape(B, C, H * W_IMG)
    x_hi = np.asarray(x, dtype=f8)
    x_lo = np.asarray(16.0 * (x - x_hi.astype(np.float32)), dtype=f8)

    W1 = np.asarray(inputs["W1"], dtype=np.float32)
    w1h16 = np.asarray(16.0 * W1, dtype=f8)
    w1h256 = np.asarray(16.0 * w1h16.astype(np.float32), dtype=f8)
    w1lo = np.asarray(256.0 * W1 - 16.0 * w1h16.astype(np.float32), dtype=f8)

    def pack_w1(w):  # [2C, HID] -> [128, 12, HID]
        return np.ascontiguousarray(
            w.reshape(2, KC, 128, HID).transpose(2, 0, 1, 3)
        ).reshape(128, 2 * KC, HID)

    w1h256 = pack_w1(w1h256)
    w1h16 = pack_w1(w1h16)
    w1lo = pack_w1(w1lo)

    W2 = np.asarray(np.asarray(inputs["W2"], dtype=np.float32), dtype=bf16)
    W3 = np.asarray(np.asarray(inputs["W3"], dtype=np.float32), dtype=bf16)
    b1 = np.asarray(inputs["b1"], dtype=np.float32)
    b2 = np.asarray(inputs["b2"], dtype=np.float32)
    b3 = np.asarray(inputs["b3"], dtype=np.float32)
    pxs = np.asarray(inputs["pxs"], dtype=np.int32)
    pys = np.asarray(inputs["pys"], dtype=np.int32)
    idx_x = pxs[:, :, 0] * W_IMG + pxs[:, :, 1]     # [B, S]
    idx_y = pys[:, :, 0] * W_IMG + pys[:, :, 1]

    in_maps = []
    for c in range(N_CORES):
        sl = slice(c * BPC, (c + 1) * BPC)
        # x2 [128, 2*KC (hv-major), PAIRS*PAIRW]
        x2 = np.zeros((128, 2, KC, PAIRS, PAIRW), dtype=f8)
        for hv, arr in ((0, x_hi[sl]), (1, x_lo[sl])):
            # arr [BPC, C, S] -> [128, KC, PAIRS, S2]
            a = arr.reshape(PAIRS, 2, KC, 128, S).transpose(3, 2, 0, 1, 4)
            x2[:, hv, :, :, :S2] = a.reshape(128, KC, PAIRS, S2)
        x2 = np.ascontiguousarray(x2).reshape(128, 2 * KC, PAIRS * PAIRW)

        # gather index table [128, PAIRS*2*IDXW] int16
        idxg = np.zeros((PAIRS, 2, NIDX), dtype=np.int16)
        for br, idx in ((0, idx_x[sl]), (1, idx_y[sl])):
            a = idx.reshape(PAIRS, 2, S)
            idxg[:, br, 0:S] = a[:, 0, :]
            idxg[:, br, SPAD:SPAD + S] = a[:, 1, :] + S2 // 2
        # wrap: index i lives at (partition i%16, word i//16)
        idxw = idxg.reshape(PAIRS, 2, IDXW, 16).transpose(3, 0, 1, 2)
        idxw = np.tile(idxw.reshape(1, 16, PAIRS, 2, IDXW), (8, 1, 1, 1, 1))
        idxw = np.ascontiguousarray(idxw).reshape(128, PAIRS * 2 * IDXW)

        in_maps.append({
            "x2": x2,
            "W1h256": w1h256, "W1h16": w1h16, "W1lo": w1lo,
            "W2": W2, "W3": W3,
            "b1": b1, "b2": b2, "b3": b3,
            "idxg": idxw,
            "pxs": np.ascontiguousarray(pxs[sl].reshape(BPC, S2)),
            "pys": np.ascontiguousarray(pys[sl].reshape(BPC, S2)),
        })
    return in_maps


def _run(inputs, trace=False):
    from concourse.bass_utils import run_bass_kernel_spmd

    nc = _get_nc()
    in_maps = _make_in_maps(inputs)
    res = run_bass_kernel_spmd(
        nc, in_maps, core_ids=list(range(N_CORES)), trace=trace
    )
    pred = np.concatenate(
        [np.ascontiguousarray(res.results[c]["predT"].T) for c in range(N_CORES)],
        axis=0,
    )
    delta = np.concatenate(
        [res.results[c]["deltaxy"] for c in range(N_CORES)], axis=0
    )
    return (pred, delta), res


def kernel(**inputs):
    (pred, delta), _ = _run(inputs, trace=False)
    return pred, delta


# revision 8
# speedup vs baseline: 1.9050x; 1.9050x over previous
"""Trainium2 Bass kernel for nn_DenseRelativeLoc.

Data-parallel over batch: 32 images per NeuronCore x 8 cores.

v3 gather-first design:
  * Host lays out x transposed as xT [BPC*196 rows, 768 ch] in DRAM.
  * dma_gather (DMA-engine indexed gather, gpsimd-triggered SWDGE) pulls
    the sampled rows straight from DRAM and transposes them into
    [128 ch, 6 ch-chunk, samples] SBUF tiles -- one gather per
    (4-batch group, branch). No one-hot matmuls, no z materialization.
  * Projection h1T[hid, s] = W1^T @ featsT runs on gathered features
    only; both branches accumulate into the same PSUM tile, so
    h1 = relu(psum + b1) comes out of a single activation.
  * GEMM2 / GEMM3 in bf16 as before.
"""
import sys
import types

import numpy as np

B, C, H, W_IMG = 256, 768, 14, 14
S = 196          # sample count == H*W
HID = 512
OUT = 2
N_CORES = 8
BPC = B // N_CORES      # 32 batches per core
PAIRS = BPC // 2        # 16
KC = C // 128           # 6 chunks per branch
MJ = HID // 128         # 4 HID chunks
S2 = 2 * S              # 392: pair width
GB = 4                  # batches per gather group
NGRP = BPC // GB        # 8 groups
NIDX = 896              # padded idx count per gather (GB*S=784 -> %128)
IDXW = NIDX // 16       # 56


def _install_ntff_hook():
    try:
        import antenv.axon_hooks  # noqa: F401
        return
    except ImportError:
        pass
    try:
        from trn_agent_boot.trn_boot import _ntff_profile_via_ctypes
        hook = _ntff_profile_via_ctypes("/opt/axon/libaxon_pjrt.so")
    except Exception:
        hook = None
    mod = types.ModuleType("antenv.axon_hooks")
    mod.get_axon_ntff_profile_hook = lambda: hook
    sys.modules["antenv.axon_hooks"] = mod


def _build_nc():
    from contextlib import ExitStack

    import concourse.bass as bass
    import concourse.bacc as bacc
    import concourse.mybir as mybir
    import concourse.tile as tile

    dt = mybir.dt
    f32, bf16, i32, i16 = dt.float32, dt.bfloat16, dt.int32, dt.int16
    AF = mybir.ActivationFunctionType
    ALU = mybir.AluOpType

    nc = bacc.Bacc(None, target_bir_lowering=False)

    xt_t = nc.dram_tensor("xT", [BPC * S, C], bf16, kind="ExternalInput")
    w1_t = nc.dram_tensor("W1", [128, 2 * KC, HID], bf16, kind="ExternalInput")
    w2_t = nc.dram_tensor("W2", [HID, HID], bf16, kind="ExternalInput")
    w3_t = nc.dram_tensor("W3", [HID, OUT], bf16, kind="ExternalInput")
    b1_t = nc.dram_tensor("b1", [HID], f32, kind="ExternalInput")
    b2_t = nc.dram_tensor("b2", [HID], f32, kind="ExternalInput")
    b3_t = nc.dram_tensor("b3", [OUT], f32, kind="ExternalInput")
    idx_t = nc.dram_tensor("idxg", [128, NGRP * 2 * IDXW], i16,
                           kind="ExternalInput")
    pxs_t = nc.dram_tensor("pxs", [BPC, S2], i32, kind="ExternalInput")
    pys_t = nc.dram_tensor("pys", [BPC, S2], i32, kind="ExternalInput")
    pred_t = nc.dram_tensor("predT", [OUT, BPC * S], f32, kind="ExternalOutput")
    delta_t = nc.dram_tensor("deltaxy", [BPC * S, OUT], f32,
                             kind="ExternalOutput")

    with ExitStack() as ctx:
        tc = ctx.enter_context(tile.TileContext(nc))
        wpool = ctx.enter_context(tc.tile_pool(name="w", bufs=1))
        gxpool = ctx.enter_context(tc.tile_pool(name="gx", bufs=3))
        h1pool = ctx.enter_context(tc.tile_pool(name="h1", bufs=2))
        h2pool = ctx.enter_context(tc.tile_pool(name="h2", bufs=2))
        opool = ctx.enter_context(tc.tile_pool(name="op", bufs=1))
        idxpool = ctx.enter_context(tc.tile_pool(name="idx", bufs=1))
        zps = ctx.enter_context(tc.tile_pool(name="zps", bufs=1, space="PSUM"))
        hps = ctx.enter_context(tc.tile_pool(name="hps", bufs=1, space="PSUM"))

        # ---------- small tensors ----------
        idxt = idxpool.tile([128, NGRP * 2 * IDXW], i16, name="idxt",
                            tag="idxt")
        nc.sync.dma_start(idxt[:], idx_t[:, :])

        pxs_sb = idxpool.tile([BPC, S2], i32, name="pxs_sb", tag="pxs_sb")
        nc.sync.dma_start(pxs_sb[:], pxs_t[:, :])
        pys_sb = idxpool.tile([BPC, S2], i32, name="pys_sb", tag="pys_sb")
        nc.sync.dma_start(pys_sb[:], pys_t[:, :])

        ones_row = wpool.tile([1, 128], bf16, name="ones_row", tag="ones_row")
        nc.vector.memset(ones_row[:], 1.0)

        # PE warm-up so the p-state ramps before real matmuls
        wmt = hps.tile([128, 128], f32, name="warm", tag="warm")
        for _ in range(24):
            nc.tensor.matmul(wmt[:], ones_row[:], ones_row[:],
                             start=True, stop=True)

        # ---------- weights ----------
        w1sb = wpool.tile([128, 2 * KC, HID], bf16, name="w1sb", tag="w1sb")
        nc.sync.dma_start(w1sb[:], w1_t[:, :, :])
        w2b = []
        for k in range(MJ):
            wb = wpool.tile([128, HID], bf16, name=f"w2b{k}", tag=f"w2b{k}")
            nc.sync.dma_start(wb[:], w2_t[k * 128:(k + 1) * 128, :])
            w2b.append(wb)
        w3b = []
        for k in range(MJ):
            wb = wpool.tile([128, OUT], bf16, name=f"w3b{k}", tag=f"w3b{k}")
            nc.sync.dma_start(wb[:], w3_t[k * 128:(k + 1) * 128, :])
            w3b.append(wb)
        b1c, b2c = [], []
        for j in range(MJ):
            t1 = wpool.tile([128, 1], f32, name=f"b1c{j}", tag=f"b1c{j}")
            nc.sync.dma_start(t1[:], b1_t[j * 128:(j + 1) * 128])
            b1c.append(t1)
            t2 = wpool.tile([128, 1], f32, name=f"b2c{j}", tag=f"b2c{j}")
            nc.sync.dma_start(t2[:], b2_t[j * 128:(j + 1) * 128])
            b2c.append(t2)
        b3c = wpool.tile([OUT, 1], f32, name="b3c", tag="b3c")
        nc.sync.dma_start(b3c[:], b3_t[:])

        # ---------- deltaxy on gpsimd ----------
        dsub = idxpool.tile([BPC, S2], i32, name="dsub", tag="dsub")
        nc.gpsimd.tensor_tensor(dsub[:], pxs_sb[:], pys_sb[:], ALU.subtract)
        ddel = idxpool.tile([BPC, S2], f32, name="ddel", tag="ddel")
        nc.gpsimd.tensor_scalar(ddel[:], dsub[:], float(H - 1), None,
                                op0=ALU.add)
        nc.sync.dma_start(bass.AP(delta_t, 0, [[S2, BPC], [1, S2]]), ddel[:])

        pred_all = opool.tile([OUT, BPC * S], f32, name="pred_all",
                              tag="pred_all")

        # ---------- per-group gathers / per-pair compute ----------
        gx_grp = {}      # G -> (gxa, gxb) [128, KC, NIDX] bf16
        h1_pair = {}     # P -> [128, MJ, S2] bf16

        def emit_gathers(G):
            if G in gx_grp or G >= NGRP:
                return
            ga = gxpool.tile([128, KC, NIDX], bf16, name=f"gxa{G}", tag="gxa")
            gb = gxpool.tile([128, KC, NIDX], bf16, name=f"gxb{G}", tag="gxb")
            for g_out, br in ((ga, 0), (gb, 1)):
                nc.gpsimd.dma_gather(
                    g_out[:], xt_t[:, :],
                    idxt[:, (G * 2 + br) * IDXW:(G * 2 + br + 1) * IDXW],
                    num_idxs=NIDX, num_idxs_reg=NIDX, elem_size=C,
                    transpose=True,
                )
            gx_grp[G] = (ga, gb)

        def emit_proj(P):
            G, q = divmod(P, 2)
            ga, gb = gx_grp[G]
            h1 = h1pool.tile([128, MJ, S2], bf16, name=f"h1_{P}", tag="h1")
            h1_pair[P] = h1
            for j in range(MJ):
                zt = zps.tile([128, S2], f32, name=f"zt{j}_{P}", tag=f"zt{j}")
                for kt in range(2 * KC):
                    gx = ga if kt < KC else gb
                    nc.tensor.matmul(
                        zt[:],
                        w1sb[:, kt, j * 128:(j + 1) * 128],
                        gx[:, kt % KC, q * S2:(q + 1) * S2],
                        start=(kt == 0), stop=(kt == 2 * KC - 1),
                    )
                if j < 2:
                    nc.scalar.activation(h1[:, j, :], zt[:], AF.Relu,
                                         bias=b1c[j][:])
                else:
                    nc.vector.tensor_scalar(h1[:, j, :], zt[:], b1c[j][:],
                                            0.0, op0=ALU.add, op1=ALU.max)

        def emit_tail(P):
            h1 = h1_pair.pop(P)
            h2 = h2pool.tile([128, MJ, S2], bf16, name=f"h2_{P}", tag="h2")
            for j in range(MJ):
                hp = hps.tile([128, S2], f32, name=f"h2ps{j}_{P}",
                              tag=f"hps{j % 2}")
                for k in range(MJ):
                    nc.tensor.matmul(
                        hp[:],
                        w2b[k][:, j * 128:(j + 1) * 128],
                        h1[:, k, :],
                        start=(k == 0), stop=(k == MJ - 1),
                    )
                if j < 2:
                    nc.scalar.activation(h2[:, j, :], hp[:], AF.Relu,
                                         bias=b2c[j][:])
                else:
                    nc.vector.tensor_scalar(h2[:, j, :], hp[:], b2c[j][:],
                                            0.0, op0=ALU.add, op1=ALU.max)
            pp = hps.tile([OUT, S2], f32, name=f"pps_{P}", tag="pps")
            for k in range(MJ):
                nc.tensor.matmul(pp[:], w3b[k][:], h2[:, k, :],
                                 start=(k == 0), stop=(k == MJ - 1))
            nc.vector.tensor_scalar(
                pred_all[:, P * S2:(P + 1) * S2], pp[:], b3c[:], None,
                op0=ALU.add,
            )
            if P % 4 == 3:
                qd = P // 4
                nc.sync.dma_start(
                    pred_t[:, qd * 4 * S2:(qd + 1) * 4 * S2],
                    pred_all[:, qd * 4 * S2:(qd + 1) * 4 * S2],
                )

        # ---------- main loop ----------
        emit_gathers(0)
        emit_gathers(1)
        for P in range(PAIRS):
            if P % 2 == 0:
                emit_gathers(P // 2 + 2)
            if P % 2 == 1:
                gx_grp.pop(P // 2 - 1, None)
            emit_proj(P)
            if P >= 1:
                emit_tail(P - 1)
        emit_tail(PAIRS - 1)

    nc.finalize()
    return nc


_NC = None


def _get_nc():
    global _NC
    if _NC is None:
        _install_ntff_hook()
        _NC = _build_nc()
    return _NC


def _make_in_maps(inputs):
    import ml_dtypes
    bf16 = ml_dtypes.bfloat16

    x = np.asarray(inputs["x"], dtype=np.float32).reshape(B, C, H * W_IMG)
    x = np.asarray(x, dtype=bf16)

    W1 = np.asarray(np.asarray(inputs["W1"], dtype=np.float32), dtype=bf16)
    # [2C, HID] -> [128, 12, HID]
    w1p = np.ascontiguousarray(
        W1.reshape(2, KC, 128, HID).transpose(2, 0, 1, 3)
    ).reshape(128, 2 * KC, HID)

    W2 = np.asarray(np.asarray(inputs["W2"], dtype=np.float32), dtype=bf16)
    W3 = np.asarray(np.asarray(inputs["W3"], dtype=np.float32), dtype=bf16)
    b1 = np.asarray(inputs["b1"], dtype=np.float32)
    b2 = np.asarray(inputs["b2"], dtype=np.float32)
    b3 = np.asarray(inputs["b3"], dtype=np.float32)
    pxs = np.asarray(inputs["pxs"], dtype=np.int32)
    pys = np.asarray(inputs["pys"], dtype=np.int32)
    idx_x = pxs[:, :, 0] * W_IMG + pxs[:, :, 1]     # [B, S]
    idx_y = pys[:, :, 0] * W_IMG + pys[:, :, 1]

    in_maps = []
    for c in range(N_CORES):
        sl = slice(c * BPC, (c + 1) * BPC)
        # xT [BPC*S, C]
        xT = np.ascontiguousarray(
            x[sl].transpose(0, 2, 1).reshape(BPC * S, C))

        # global row indices per (group, branch), wrapped for dge
        base = (np.arange(BPC, dtype=np.int32) * S)[:, None]   # [BPC, 1]
        gidx = np.zeros((NGRP, 2, NIDX), dtype=np.int16)
        for br, idx in ((0, idx_x[sl]), (1, idx_y[sl])):
            gl = (idx + base).astype(np.int16).reshape(NGRP, GB * S)
            gidx[:, br, :GB * S] = gl
        idxw = gidx.reshape(NGRP, 2, IDXW, 16).transpose(3, 0, 1, 2)
        idxw = np.tile(idxw.reshape(1, 16, NGRP, 2, IDXW), (8, 1, 1, 1, 1))
        idxw = np.ascontiguousarray(idxw).reshape(128, NGRP * 2 * IDXW)

        in_maps.append({
            "xT": xT,
            "W1": w1p, "W2": W2, "W3": W3,
            "b1": b1, "b2": b2, "b3": b3,
            "idxg": idxw,
            "pxs": np.ascontiguousarray(pxs[sl].reshape(BPC, S2)),
            "pys": np.ascontiguousarray(pys[sl].reshape(BPC, S2)),
        })
    return in_maps


def _run(inputs, trace=False):
    from concourse.bass_utils import run_bass_kernel_spmd

    nc = _get_nc()
    in_maps = _make_in_maps(inputs)
    res = run_bass_kernel_spmd(
        nc, in_maps, core_ids=list(range(N_CORES)), trace=trace
    )
    pred = np.concatenate(
        [np.ascontiguousarray(res.results[c]["predT"].T) for c in range(N_CORES)],
        axis=0,
    )
    delta = np.concatenate(
        [res.results[c]["deltaxy"] for c in range(N_CORES)], axis=0
    )
    return (pred, delta), res


def kernel(**inputs):
    (pred, delta), _ = _run(inputs, trace=False)
    return pred, delta


# revision 11
# speedup vs baseline: 2.2479x; 1.1800x over previous
"""Trainium2 Bass kernel for nn_DenseRelativeLoc.

Data-parallel over batch: 32 images per NeuronCore x 8 cores.

v3 gather-first design:
  * Host lays out x transposed as xT [BPC*196 rows, 768 ch] in DRAM.
  * dma_gather (DMA-engine indexed gather, gpsimd-triggered SWDGE) pulls
    the sampled rows straight from DRAM and transposes them into
    [128 ch, 6 ch-chunk, samples] SBUF tiles -- one gather per
    (4-batch group, branch). No one-hot matmuls, no z materialization.
  * Projection h1T[hid, s] = W1^T @ featsT runs on gathered features
    only; both branches accumulate into the same PSUM tile, so
    h1 = relu(psum + b1) comes out of a single activation.
  * GEMM2 / GEMM3 in bf16 as before.
"""
import sys
import types

import numpy as np

B, C, H, W_IMG = 256, 768, 14, 14
S = 196          # sample count == H*W
HID = 512
OUT = 2
N_CORES = 8
BPC = B // N_CORES      # 32 batches per core
PAIRS = BPC // 2        # 16
KC = C // 128           # 6 chunks per branch
MJ = HID // 128         # 4 HID chunks
S2 = 2 * S              # 392: pair width
GB = 4                  # batches per gather group
NGRP = BPC // GB        # 8 groups
NIDX = 896              # padded idx count per gather (GB*S=784 -> %128)
IDXW = NIDX // 16       # 56


def _install_ntff_hook():
    try:
        import antenv.axon_hooks  # noqa: F401
        return
    except ImportError:
        pass
    try:
        from trn_agent_boot.trn_boot import _ntff_profile_via_ctypes
        hook = _ntff_profile_via_ctypes("/opt/axon/libaxon_pjrt.so")
    except Exception:
        hook = None
    mod = types.ModuleType("antenv.axon_hooks")
    mod.get_axon_ntff_profile_hook = lambda: hook
    sys.modules["antenv.axon_hooks"] = mod


def _build_nc():
    from contextlib import ExitStack

    import concourse.bass as bass
    import concourse.bacc as bacc
    import concourse.mybir as mybir
    import concourse.tile as tile

    dt = mybir.dt
    f32, bf16, i32, i16 = dt.float32, dt.bfloat16, dt.int32, dt.int16
    AF = mybir.ActivationFunctionType
    ALU = mybir.AluOpType

    nc = bacc.Bacc(None, target_bir_lowering=False)

    xt_t = nc.dram_tensor("xT", [BPC * S, C], bf16, kind="ExternalInput")
    w1_t = nc.dram_tensor("W1", [128, 2 * KC, HID], bf16, kind="ExternalInput")
    w2_t = nc.dram_tensor("W2", [HID, HID], bf16, kind="ExternalInput")
    w3_t = nc.dram_tensor("W3", [HID, OUT], bf16, kind="ExternalInput")
    b1_t = nc.dram_tensor("b1", [HID], f32, kind="ExternalInput")
    b2_t = nc.dram_tensor("b2", [HID], f32, kind="ExternalInput")
    b3_t = nc.dram_tensor("b3", [OUT], f32, kind="ExternalInput")
    idx_t = nc.dram_tensor("idxg", [128, NGRP * 2 * IDXW], i16,
                           kind="ExternalInput")
    pxs_t = nc.dram_tensor("pxs", [BPC, S2], i32, kind="ExternalInput")
    pys_t = nc.dram_tensor("pys", [BPC, S2], i32, kind="ExternalInput")
    pred_t = nc.dram_tensor("predT", [OUT, BPC * S], f32, kind="ExternalOutput")
    delta_t = nc.dram_tensor("deltaxy", [BPC * S, OUT], f32,
                             kind="ExternalOutput")

    with ExitStack() as ctx:
        tc = ctx.enter_context(tile.TileContext(nc))
        wpool = ctx.enter_context(tc.tile_pool(name="w", bufs=1))
        gxpool = ctx.enter_context(tc.tile_pool(name="gx", bufs=3))
        h1pool = ctx.enter_context(tc.tile_pool(name="h1", bufs=2))
        h2pool = ctx.enter_context(tc.tile_pool(name="h2", bufs=2))
        opool = ctx.enter_context(tc.tile_pool(name="op", bufs=1))
        idxpool = ctx.enter_context(tc.tile_pool(name="idx", bufs=1))
        zps = ctx.enter_context(tc.tile_pool(name="zps", bufs=1, space="PSUM"))
        hps = ctx.enter_context(tc.tile_pool(name="hps", bufs=1, space="PSUM"))

        # ---------- small tensors ----------
        idxt = idxpool.tile([128, NGRP * 2 * IDXW], i16, name="idxt",
                            tag="idxt")
        nc.sync.dma_start(idxt[:], idx_t[:, :])

        pxs_sb = idxpool.tile([BPC, S2], i32, name="pxs_sb", tag="pxs_sb")
        nc.sync.dma_start(pxs_sb[:], pxs_t[:, :])
        pys_sb = idxpool.tile([BPC, S2], i32, name="pys_sb", tag="pys_sb")
        nc.sync.dma_start(pys_sb[:], pys_t[:, :])

        ones_row = wpool.tile([1, 128], bf16, name="ones_row", tag="ones_row")
        nc.vector.memset(ones_row[:], 1.0)

        # PE warm-up so the p-state ramps before real matmuls
        wmt = hps.tile([128, 128], f32, name="warm", tag="warm")
        for _ in range(24):
            nc.tensor.matmul(wmt[:], ones_row[:], ones_row[:],
                             start=True, stop=True)

        # ---------- weights ----------
        w1sb = wpool.tile([128, 2 * KC, HID], bf16, name="w1sb", tag="w1sb")
        nc.sync.dma_start(w1sb[:], w1_t[:, :, :])
        w2b = []
        for k in range(MJ):
            wb = wpool.tile([128, HID], bf16, name=f"w2b{k}", tag=f"w2b{k}")
            nc.sync.dma_start(wb[:], w2_t[k * 128:(k + 1) * 128, :])
            w2b.append(wb)
        w3b = []
        for k in range(MJ):
            wb = wpool.tile([128, OUT], bf16, name=f"w3b{k}", tag=f"w3b{k}")
            nc.sync.dma_start(wb[:], w3_t[k * 128:(k + 1) * 128, :])
            w3b.append(wb)
        b1c, b2c = [], []
        for j in range(MJ):
            t1 = wpool.tile([128, 1], f32, name=f"b1c{j}", tag=f"b1c{j}")
            nc.sync.dma_start(t1[:], b1_t[j * 128:(j + 1) * 128])
            b1c.append(t1)
            t2 = wpool.tile([128, 1], f32, name=f"b2c{j}", tag=f"b2c{j}")
            nc.sync.dma_start(t2[:], b2_t[j * 128:(j + 1) * 128])
            b2c.append(t2)
        b3c = wpool.tile([OUT, 1], f32, name="b3c", tag="b3c")
        nc.sync.dma_start(b3c[:], b3_t[:])

        # ---------- deltaxy on gpsimd ----------
        dsub = idxpool.tile([BPC, S2], i32, name="dsub", tag="dsub")
        nc.gpsimd.tensor_tensor(dsub[:], pxs_sb[:], pys_sb[:], ALU.subtract)
        ddel = idxpool.tile([BPC, S2], f32, name="ddel", tag="ddel")
        nc.gpsimd.tensor_scalar(ddel[:], dsub[:], float(H - 1), None,
                                op0=ALU.add)
        nc.sync.dma_start(bass.AP(delta_t, 0, [[S2, BPC], [1, S2]]), ddel[:])

        pred_all = opool.tile([OUT, BPC * S], f32, name="pred_all",
                              tag="pred_all")

        # ---------- per-group gathers / per-pair compute ----------
        gx_grp = {}      # G -> (gxa, gxb) [128, KC, NIDX] bf16
        h1_pair = {}     # P -> [128, MJ, S2] bf16

        def emit_gathers(G):
            if G in gx_grp or G >= NGRP:
                return
            ga = gxpool.tile([128, KC, NIDX], bf16, name=f"gxa{G}", tag="gxa")
            gb = gxpool.tile([128, KC, NIDX], bf16, name=f"gxb{G}", tag="gxb")
            for g_out, br in ((ga, 0), (gb, 1)):
                nc.gpsimd.dma_gather(
                    g_out[:], xt_t[:, :],
                    idxt[:, (G * 2 + br) * IDXW:(G * 2 + br + 1) * IDXW],
                    num_idxs=NIDX, num_idxs_reg=NIDX, elem_size=C,
                    transpose=True,
                )
            gx_grp[G] = (ga, gb)

        def emit_proj(P):
            G, q = divmod(P, 2)
            ga, gb = gx_grp[G]
            h1 = h1pool.tile([128, MJ, S2], bf16, name=f"h1_{P}", tag="h1")
            h1_pair[P] = h1
            for j in range(MJ):
                zt = zps.tile([128, S2], f32, name=f"zt{j}_{P}", tag=f"zt{j}")
                for kt in range(2 * KC):
                    gx = ga if kt < KC else gb
                    nc.tensor.matmul(
                        zt[:],
                        w1sb[:, kt, j * 128:(j + 1) * 128],
                        gx[:, kt % KC, q * S2:(q + 1) * S2],
                        start=(kt == 0), stop=(kt == 2 * KC - 1),
                    )
                if j < 2:
                    nc.scalar.activation(h1[:, j, :], zt[:], AF.Relu,
                                         bias=b1c[j][:])
                else:
                    nc.vector.tensor_scalar(h1[:, j, :], zt[:], b1c[j][:],
                                            0.0, op0=ALU.add, op1=ALU.max)

        def emit_tail(P):
            h1 = h1_pair.pop(P)
            h2 = h2pool.tile([128, MJ, S2], bf16, name=f"h2_{P}", tag="h2")
            for j in range(MJ):
                hp = hps.tile([128, S2], f32, name=f"h2ps{j}_{P}",
                              tag=f"hps{j % 2}")
                for k in range(MJ):
                    nc.tensor.matmul(
                        hp[:],
                        w2b[k][:, j * 128:(j + 1) * 128],
                        h1[:, k, :],
                        start=(k == 0), stop=(k == MJ - 1),
                    )
                if j < 2:
                    nc.scalar.activation(h2[:, j, :], hp[:], AF.Relu,
                                         bias=b2c[j][:])
                else:
                    nc.vector.tensor_scalar(h2[:, j, :], hp[:], b2c[j][:],
                                            0.0, op0=ALU.add, op1=ALU.max)
            pp = hps.tile([OUT, S2], f32, name=f"pps_{P}", tag="pps")
            for k in range(MJ):
                nc.tensor.matmul(pp[:], w3b[k][:], h2[:, k, :],
                                 start=(k == 0), stop=(k == MJ - 1))
            nc.vector.tensor_scalar(
                pred_all[:, P * S2:(P + 1) * S2], pp[:], b3c[:], None,
                op0=ALU.add,
            )
            if P % 4 == 3:
                qd = P // 4
                nc.sync.dma_start(
                    pred_t[:, qd * 4 * S2:(qd + 1) * 4 * S2],
                    pred_all[:, qd * 4 * S2:(qd + 1) * 4 * S2],
                )

        # ---------- main loop ----------
        emit_gathers(0)
        emit_gathers(1)
        for P in range(PAIRS):
            if P % 2 == 0:
                emit_gathers(P // 2 + 2)
            if P % 2 == 1:
                gx_grp.pop(P // 2 - 1, None)
            emit_proj(P)
            if P >= 1:
                emit_tail(P - 1)
        emit_tail(PAIRS - 1)

    nc.finalize()
    return nc


_NC = None


def _get_nc():
    global _NC
    if _NC is None:
        _install_ntff_hook()
        _NC = _build_nc()
    return _NC


def _make_in_maps(inputs):
    import ml_dtypes
    bf16 = ml_dtypes.bfloat16

    x = np.asarray(inputs["x"], dtype=np.float32).reshape(B, C, H * W_IMG)
    x = np.asarray(x, dtype=bf16)

    W1 = np.asarray(np.asarray(inputs["W1"], dtype=np.float32), dtype=bf16)
    # [2C, HID] -> [128, 12, HID]
    w1p = np.ascontiguousarray(
        W1.reshape(2, KC, 128, HID).transpose(2, 0, 1, 3)
    ).reshape(128, 2 * KC, HID)

    W2 = np.asarray(np.asarray(inputs["W2"], dtype=np.float32), dtype=bf16)
    W3 = np.asarray(np.asarray(inputs["W3"], dtype=np.float32), dtype=bf16)
    b1 = np.asarray(inputs["b1"], dtype=np.float32)
    b2 = np.asarray(inputs["b2"], dtype=np.float32)
    b3 = np.asarray(inputs["b3"], dtype=np.float32)
    pxs = np.asarray(inputs["pxs"], dtype=np.int32)
    pys = np.asarray(inputs["pys"], dtype=np.int32)
    idx_x = pxs[:, :, 0] * W_IMG + pxs[:, :, 1]     # [B, S]
    idx_y = pys[:, :, 0] * W_IMG + pys[:, :, 1]

    in_maps = []
    for c in range(N_CORES):
        sl = slice(c * BPC, (c + 1) * BPC)
        # xT [BPC*S, C]
        xT = np.ascontiguousarray(
            x[sl].transpose(0, 2, 1).reshape(BPC * S, C))

        # global row indices per (group, branch), wrapped for dge
        base = (np.arange(BPC, dtype=np.int32) * S)[:, None]   # [BPC, 1]
        gidx = np.zeros((NGRP, 2, NIDX), dtype=np.int16)
        for br, idx in ((0, idx_x[sl]), (1, idx_y[sl])):
            gl = (idx + base).astype(np.int16).reshape(NGRP, GB * S)
            gidx[:, br, :GB * S] = gl
        idxw = gidx.reshape(NGRP, 2, IDXW, 16).transpose(3, 0, 1, 2)
        idxw = np.tile(idxw.reshape(1, 16, NGRP, 2, IDXW), (8, 1, 1, 1, 1))
        idxw = np.ascontiguousarray(idxw).reshape(128, NGRP * 2 * IDXW)

        in_maps.append({
            "xT": xT,
            "W1": w1p, "W2": W2, "W3": W3,
            "b1": b1, "b2": b2, "b3": b3,
            "idxg": idxw,
            "pxs": np.ascontiguousarray(pxs[sl].reshape(BPC, S2)),
            "pys": np.ascontiguousarray(pys[sl].reshape(BPC, S2)),
        })
    return in_maps


def _run(inputs, trace=False):
    from concourse.bass_utils import run_bass_kernel_spmd

    nc = _get_nc()
    in_maps = _make_in_maps(inputs)
    res = run_bass_kernel_spmd(
        nc, in_maps, core_ids=list(range(N_CORES)), trace=trace
    )
    pred = np.concatenate(
        [np.ascontiguousarray(res.results[c]["predT"].T) for c in range(N_CORES)],
        axis=0,
    )
    delta = np.concatenate(
        [res.results[c]["deltaxy"] for c in range(N_CORES)], axis=0
    )
    return (pred, delta), res


def kernel(**inputs):
    (pred, delta), _ = _run(inputs, trace=False)
    return pred, delta


# revision 16
# speedup vs baseline: 2.4240x; 1.0783x over previous
"""Trainium2 Bass kernel for nn_DenseRelativeLoc.

Data-parallel over batch: 32 images per NeuronCore x 8 cores.

v3 gather-first design:
  * Host lays out x transposed as xT [BPC*196 rows, 768 ch] in DRAM.
  * dma_gather (DMA-engine indexed gather, gpsimd-triggered SWDGE) pulls
    the sampled rows straight from DRAM and transposes them into
    [128 ch, 6 ch-chunk, samples] SBUF tiles -- one gather per
    (4-batch group, branch). No one-hot matmuls, no z materialization.
  * Projection h1T[hid, s] = W1^T @ featsT runs on gathered features
    only; both branches accumulate into the same PSUM tile, so
    h1 = relu(psum + b1) comes out of a single activation.
  * GEMM2 / GEMM3 in bf16 as before.
"""
import sys
import types

import numpy as np

B, C, H, W_IMG = 256, 768, 14, 14
S = 196          # sample count == H*W
HID = 512
OUT = 2
N_CORES = 8
BPC = B // N_CORES      # 32 batches per core
PAIRS = BPC // 2        # 16
KC = C // 128           # 6 chunks per branch
MJ = HID // 128         # 4 HID chunks
S2 = 2 * S              # 392: pair width
GB = 4                  # batches per gather group
NGRP = BPC // GB        # 8 groups
NIDX = 896              # padded idx count per gather (GB*S=784 -> %128)
IDXW = NIDX // 16       # 56


def _install_ntff_hook():
    try:
        import antenv.axon_hooks  # noqa: F401
        return
    except ImportError:
        pass
    try:
        from trn_agent_boot.trn_boot import _ntff_profile_via_ctypes
        hook = _ntff_profile_via_ctypes("/opt/axon/libaxon_pjrt.so")
    except Exception:
        hook = None
    mod = types.ModuleType("antenv.axon_hooks")
    mod.get_axon_ntff_profile_hook = lambda: hook
    sys.modules["antenv.axon_hooks"] = mod


def _build_nc():
    from contextlib import ExitStack

    import concourse.bass as bass
    import concourse.bacc as bacc
    import concourse.mybir as mybir
    import concourse.tile as tile

    dt = mybir.dt
    f32, bf16, i32, i16 = dt.float32, dt.bfloat16, dt.int32, dt.int16
    AF = mybir.ActivationFunctionType
    ALU = mybir.AluOpType

    nc = bacc.Bacc(None, target_bir_lowering=False)

    xt_t = nc.dram_tensor("xT", [BPC * S, C], bf16, kind="ExternalInput")
    w1_t = nc.dram_tensor("W1", [128, 2 * KC, HID], bf16, kind="ExternalInput")
    w2_t = nc.dram_tensor("W2", [HID, HID], bf16, kind="ExternalInput")
    w3_t = nc.dram_tensor("W3", [HID, OUT], bf16, kind="ExternalInput")
    b1_t = nc.dram_tensor("b1", [HID], f32, kind="ExternalInput")
    b2_t = nc.dram_tensor("b2", [HID], f32, kind="ExternalInput")
    b3_t = nc.dram_tensor("b3", [OUT], f32, kind="ExternalInput")
    idx_t = nc.dram_tensor("idxg", [128, NGRP * 2 * IDXW], i16,
                           kind="ExternalInput")
    pxs_t = nc.dram_tensor("pxs", [BPC, S2], i32, kind="ExternalInput")
    pys_t = nc.dram_tensor("pys", [BPC, S2], i32, kind="ExternalInput")
    pred_t = nc.dram_tensor("predT", [OUT, BPC * S], f32, kind="ExternalOutput")
    delta_t = nc.dram_tensor("deltaxy", [BPC * S, OUT], f32,
                             kind="ExternalOutput")

    with ExitStack() as ctx:
        tc = ctx.enter_context(tile.TileContext(nc))
        wpool = ctx.enter_context(tc.tile_pool(name="w", bufs=1))
        gxpool = ctx.enter_context(tc.tile_pool(name="gx", bufs=3))
        h1pool = ctx.enter_context(tc.tile_pool(name="h1", bufs=2))
        h2pool = ctx.enter_context(tc.tile_pool(name="h2", bufs=2))
        opool = ctx.enter_context(tc.tile_pool(name="op", bufs=1))
        idxpool = ctx.enter_context(tc.tile_pool(name="idx", bufs=1))
        zps = ctx.enter_context(tc.tile_pool(name="zps", bufs=1, space="PSUM"))
        hps = ctx.enter_context(tc.tile_pool(name="hps", bufs=1, space="PSUM"))

        # ---------- small tensors ----------
        idxt = idxpool.tile([128, NGRP * 2 * IDXW], i16, name="idxt",
                            tag="idxt")
        nc.sync.dma_start(idxt[:], idx_t[:, :])

        # ---------- gathers first: keep the gpsimd/SWDGE queue clear ------
        gx_grp = {}      # G -> (gxa, gxb) [128, KC, NIDX] bf16

        def emit_gathers(G):
            if G in gx_grp or G >= NGRP:
                return
            ga = gxpool.tile([128, KC, NIDX], bf16, name=f"gxa{G}", tag="gxa")
            gb = gxpool.tile([128, KC, NIDX], bf16, name=f"gxb{G}", tag="gxb")
            for g_out, br in ((ga, 0), (gb, 1)):
                nc.gpsimd.dma_gather(
                    g_out[:], xt_t[:, :],
                    idxt[:, (G * 2 + br) * IDXW:(G * 2 + br + 1) * IDXW],
                    num_idxs=NIDX, num_idxs_reg=NIDX, elem_size=C,
                    transpose=True,
                )
            gx_grp[G] = (ga, gb)

        emit_gathers(0)
        emit_gathers(1)
        emit_gathers(2)

        pxs_sb = idxpool.tile([BPC, S2], i32, name="pxs_sb", tag="pxs_sb")
        nc.sync.dma_start(pxs_sb[:], pxs_t[:, :])
        pys_sb = idxpool.tile([BPC, S2], i32, name="pys_sb", tag="pys_sb")
        nc.sync.dma_start(pys_sb[:], pys_t[:, :])

        ones_row = wpool.tile([1, 128], bf16, name="ones_row", tag="ones_row")
        nc.vector.memset(ones_row[:], 1.0)

        # PE warm-up so the p-state ramps before real matmuls
        wmt = hps.tile([128, 128], f32, name="warm", tag="warm")
        for _ in range(48):
            nc.tensor.matmul(wmt[:], ones_row[:], ones_row[:],
                             start=True, stop=True)

        # ---------- weights ----------
        w1sb = wpool.tile([128, 2 * KC, HID], bf16, name="w1sb", tag="w1sb")
        nc.sync.dma_start(w1sb[:], w1_t[:, :, :])
        w2b = []
        for k in range(MJ):
            wb = wpool.tile([128, HID], bf16, name=f"w2b{k}", tag=f"w2b{k}")
            nc.sync.dma_start(wb[:], w2_t[k * 128:(k + 1) * 128, :])
            w2b.append(wb)
        w3b = []
        for k in range(MJ):
            wb = wpool.tile([128, OUT], bf16, name=f"w3b{k}", tag=f"w3b{k}")
            nc.sync.dma_start(wb[:], w3_t[k * 128:(k + 1) * 128, :])
            w3b.append(wb)
        b1c, b2c = [], []
        for j in range(MJ):
            t1 = wpool.tile([128, 1], f32, name=f"b1c{j}", tag=f"b1c{j}")
            nc.sync.dma_start(t1[:], b1_t[j * 128:(j + 1) * 128])
            b1c.append(t1)
            t2 = wpool.tile([128, 1], f32, name=f"b2c{j}", tag=f"b2c{j}")
            nc.sync.dma_start(t2[:], b2_t[j * 128:(j + 1) * 128])
            b2c.append(t2)
        b3c = wpool.tile([OUT, 1], f32, name="b3c", tag="b3c")
        nc.sync.dma_start(b3c[:], b3_t[:])

        # ---------- deltaxy on vector (gpsimd stays free for gathers) -----
        dsub = idxpool.tile([BPC, S2], i32, name="dsub", tag="dsub")
        nc.vector.tensor_tensor(dsub[:], pxs_sb[:], pys_sb[:], ALU.subtract)
        ddel = idxpool.tile([BPC, S2], f32, name="ddel", tag="ddel")
        nc.vector.tensor_scalar(ddel[:], dsub[:], float(H - 1), None,
                                op0=ALU.add)
        nc.sync.dma_start(bass.AP(delta_t, 0, [[S2, BPC], [1, S2]]), ddel[:])

        pred_all = opool.tile([OUT, BPC * S], f32, name="pred_all",
                              tag="pred_all")

        # ---------- per-pair compute ----------
        h1_pair = {}     # P -> [128, MJ, S2] bf16

        def emit_proj(P):
            G, q = divmod(P, 2)
            ga, gb = gx_grp[G]
            h1 = h1pool.tile([128, MJ, S2], bf16, name=f"h1_{P}", tag="h1")
            h1_pair[P] = h1
            for j in range(MJ):
                zt = zps.tile([128, S2], f32, name=f"zt{j}_{P}", tag=f"zt{j}")
                for kt in range(2 * KC):
                    gx = ga if kt < KC else gb
                    nc.tensor.matmul(
                        zt[:],
                        w1sb[:, kt, j * 128:(j + 1) * 128],
                        gx[:, kt % KC, q * S2:(q + 1) * S2],
                        start=(kt == 0), stop=(kt == 2 * KC - 1),
                    )
                if j < 2:
                    nc.scalar.activation(h1[:, j, :], zt[:], AF.Relu,
                                         bias=b1c[j][:])
                else:
                    nc.vector.tensor_scalar(h1[:, j, :], zt[:], b1c[j][:],
                                            0.0, op0=ALU.add, op1=ALU.max)

        def emit_tail(P):
            h1 = h1_pair.pop(P)
            h2 = h2pool.tile([128, MJ, S2], bf16, name=f"h2_{P}", tag="h2")
            for j in range(MJ):
                hp = hps.tile([128, S2], f32, name=f"h2ps{j}_{P}",
                              tag=f"hps{j % 2}")
                for k in range(MJ):
                    nc.tensor.matmul(
                        hp[:],
                        w2b[k][:, j * 128:(j + 1) * 128],
                        h1[:, k, :],
                        start=(k == 0), stop=(k == MJ - 1),
                    )
                if j < 2:
                    nc.scalar.activation(h2[:, j, :], hp[:], AF.Relu,
                                         bias=b2c[j][:])
                else:
                    nc.vector.tensor_scalar(h2[:, j, :], hp[:], b2c[j][:],
                                            0.0, op0=ALU.add, op1=ALU.max)
            pp = hps.tile([OUT, S2], f32, name=f"pps_{P}", tag="pps")
            for k in range(MJ):
                nc.tensor.matmul(pp[:], w3b[k][:], h2[:, k, :],
                                 start=(k == 0), stop=(k == MJ - 1))
            nc.vector.tensor_scalar(
                pred_all[:, P * S2:(P + 1) * S2], pp[:], b3c[:], None,
                op0=ALU.add,
            )
            if P % 4 == 3:
                qd = P // 4
                nc.sync.dma_start(
                    pred_t[:, qd * 4 * S2:(qd + 1) * 4 * S2],
                    pred_all[:, qd * 4 * S2:(qd + 1) * 4 * S2],
                )

        # ---------- main loop ----------
        for P in range(PAIRS):
            if P % 2 == 0:
                emit_gathers(P // 2 + 3)
            if P % 2 == 1:
                gx_grp.pop(P // 2 - 1, None)
            emit_proj(P)
            if P >= 1:
                emit_tail(P - 1)
        emit_tail(PAIRS - 1)

    nc.finalize()
    return nc


_NC = None


def _get_nc():
    global _NC
    if _NC is None:
        _install_ntff_hook()
        _NC = _build_nc()
    return _NC


def _make_in_maps(inputs):
    import ml_dtypes
    bf16 = ml_dtypes.bfloat16

    x = np.asarray(inputs["x"], dtype=np.float32).reshape(B, C, H * W_IMG)
    x = np.asarray(x, dtype=bf16)

    W1 = np.asarray(np.asarray(inputs["W1"], dtype=np.float32), dtype=bf16)
    # [2C, HID] -> [128, 12, HID]
    w1p = np.ascontiguousarray(
        W1.reshape(2, KC, 128, HID).transpose(2, 0, 1, 3)
    ).reshape(128, 2 * KC, HID)

    W2 = np.asarray(np.asarray(inputs["W2"], dtype=np.float32), dtype=bf16)
    W3 = np.asarray(np.asarray(inputs["W3"], dtype=np.float32), dtype=bf16)
    b1 = np.asarray(inputs["b1"], dtype=np.float32)
    b2 = np.asarray(inputs["b2"], dtype=np.float32)
    b3 = np.asarray(inputs["b3"], dtype=np.float32)
    pxs = np.asarray(inputs["pxs"], dtype=np.int32)
    pys = np.asarray(inputs["pys"], dtype=np.int32)
    idx_x = pxs[:, :, 0] * W_IMG + pxs[:, :, 1]     # [B, S]
    idx_y = pys[:, :, 0] * W_IMG + pys[:, :, 1]

    in_maps = []
    for c in range(N_CORES):
        sl = slice(c * BPC, (c + 1) * BPC)
        # xT [BPC*S, C]
        xT = np.ascontiguousarray(
            x[sl].transpose(0, 2, 1).reshape(BPC * S, C))

        # global row indices per (group, branch), wrapped for dge
        base = (np.arange(BPC, dtype=np.int32) * S)[:, None]   # [BPC, 1]
        gidx = np.zeros((NGRP, 2, NIDX), dtype=np.int16)
        for br, idx in ((0, idx_x[sl]), (1, idx_y[sl])):
            gl = (idx + base).astype(np.int16).reshape(NGRP, GB * S)
            gidx[:, br, :GB * S] = gl
        idxw = gidx.reshape(NGRP, 2, IDXW, 16).transpose(3, 0, 1, 2)
        idxw = np.tile(idxw.reshape(1, 16, NGRP, 2, IDXW), (8, 1, 1, 1, 1))
        idxw = np.ascontiguousarray(idxw).reshape(128, NGRP * 2 * IDXW)

        in_maps.append({
            "xT": xT,
            "W1": w1p, "W2": W2, "W3": W3,
            "b1": b1, "b2": b2, "b3": b3,
            "idxg": idxw,
            "pxs": np.ascontiguousarray(pxs[sl].reshape(BPC, S2)),
            "pys": np.ascontiguousarray(pys[sl].reshape(BPC, S2)),
        })
    return in_maps


def _run(inputs, trace=False):
    from concourse.bass_utils import run_bass_kernel_spmd

    nc = _get_nc()
    in_maps = _make_in_maps(inputs)
    res = run_bass_kernel_spmd(
        nc, in_maps, core_ids=list(range(N_CORES)), trace=trace
    )
    pred = np.concatenate(
        [np.ascontiguousarray(res.results[c]["predT"].T) for c in range(N_CORES)],
        axis=0,
    )
    delta = np.concatenate(
        [res.results[c]["deltaxy"] for c in range(N_CORES)], axis=0
    )
    return (pred, delta), res


def kernel(**inputs):
    (pred, delta), _ = _run(inputs, trace=False)
    return pred, delta


# revision 18
# speedup vs baseline: 2.4815x; 1.0237x over previous
"""Trainium2 Bass kernel for nn_DenseRelativeLoc.

Data-parallel over batch: 32 images per NeuronCore x 8 cores.

v3 gather-first design:
  * Host lays out x transposed as xT [BPC*196 rows, 768 ch] in DRAM.
  * dma_gather (DMA-engine indexed gather, gpsimd-triggered SWDGE) pulls
    the sampled rows straight from DRAM and transposes them into
    [128 ch, 6 ch-chunk, samples] SBUF tiles -- one gather per
    (4-batch group, branch). No one-hot matmuls, no z materialization.
  * Projection h1T[hid, s] = W1^T @ featsT runs on gathered features
    only; both branches accumulate into the same PSUM tile, so
    h1 = relu(psum + b1) comes out of a single activation.
  * GEMM2 / GEMM3 in bf16 as before.
"""
import sys
import types

import numpy as np

B, C, H, W_IMG = 256, 768, 14, 14
S = 196          # sample count == H*W
HID = 512
OUT = 2
N_CORES = 8
BPC = B // N_CORES      # 32 batches per core
PAIRS = BPC // 2        # 16
KC = C // 128           # 6 chunks per branch
MJ = HID // 128         # 4 HID chunks
S2 = 2 * S              # 392: pair width
GB = 4                  # batches per gather group
NGRP = BPC // GB        # 8 groups
NIDX = 896              # padded idx count per gather (GB*S=784 -> %128)
IDXW = NIDX // 16       # 56


def _install_ntff_hook():
    try:
        import antenv.axon_hooks  # noqa: F401
        return
    except ImportError:
        pass
    try:
        from trn_agent_boot.trn_boot import _ntff_profile_via_ctypes
        hook = _ntff_profile_via_ctypes("/opt/axon/libaxon_pjrt.so")
    except Exception:
        hook = None
    mod = types.ModuleType("antenv.axon_hooks")
    mod.get_axon_ntff_profile_hook = lambda: hook
    sys.modules["antenv.axon_hooks"] = mod


def _build_nc():
    from contextlib import ExitStack

    import concourse.bass as bass
    import concourse.bacc as bacc
    import concourse.mybir as mybir
    import concourse.tile as tile

    dt = mybir.dt
    f32, bf16, i32, i16 = dt.float32, dt.bfloat16, dt.int32, dt.int16
    AF = mybir.ActivationFunctionType
    ALU = mybir.AluOpType

    nc = bacc.Bacc(None, target_bir_lowering=False)

    xt_t = nc.dram_tensor("xT", [BPC * S, C], bf16, kind="ExternalInput")
    w1_t = nc.dram_tensor("W1", [128, 2 * KC, HID], bf16, kind="ExternalInput")
    w2_t = nc.dram_tensor("W2", [HID, HID], bf16, kind="ExternalInput")
    w3_t = nc.dram_tensor("W3", [HID, OUT], bf16, kind="ExternalInput")
    b1_t = nc.dram_tensor("b1", [HID], f32, kind="ExternalInput")
    b2_t = nc.dram_tensor("b2", [HID], f32, kind="ExternalInput")
    b3_t = nc.dram_tensor("b3", [OUT], f32, kind="ExternalInput")
    idx_t = nc.dram_tensor("idxg", [128, NGRP * 2 * IDXW], i16,
                           kind="ExternalInput")
    pxs_t = nc.dram_tensor("pxs", [BPC, S2], i32, kind="ExternalInput")
    pys_t = nc.dram_tensor("pys", [BPC, S2], i32, kind="ExternalInput")
    pred_t = nc.dram_tensor("predT", [OUT, BPC * S], f32, kind="ExternalOutput")
    delta_t = nc.dram_tensor("deltaxy", [BPC * S, OUT], f32,
                             kind="ExternalOutput")

    with ExitStack() as ctx:
        tc = ctx.enter_context(tile.TileContext(nc))
        wpool = ctx.enter_context(tc.tile_pool(name="w", bufs=1))
        gxpool = ctx.enter_context(tc.tile_pool(name="gx", bufs=3))
        h1pool = ctx.enter_context(tc.tile_pool(name="h1", bufs=2))
        h2pool = ctx.enter_context(tc.tile_pool(name="h2", bufs=2))
        opool = ctx.enter_context(tc.tile_pool(name="op", bufs=1))
        idxpool = ctx.enter_context(tc.tile_pool(name="idx", bufs=1))
        zps = ctx.enter_context(tc.tile_pool(name="zps", bufs=1, space="PSUM"))
        hps = ctx.enter_context(tc.tile_pool(name="hps", bufs=1, space="PSUM"))

        # ---------- small tensors ----------
        idxt = idxpool.tile([128, NGRP * 2 * IDXW], i16, name="idxt",
                            tag="idxt")
        nc.sync.dma_start(idxt[:], idx_t[:, :])

        # ---------- gathers first: keep the gpsimd/SWDGE queue clear ------
        gx_grp = {}      # G -> (gxa, gxb) [128, KC, NIDX] bf16

        def emit_gathers(G):
            if G in gx_grp or G >= NGRP:
                return
            ga = gxpool.tile([128, KC, NIDX], bf16, name=f"gxa{G}", tag="gxa")
            gb = gxpool.tile([128, KC, NIDX], bf16, name=f"gxb{G}", tag="gxb")
            for g_out, br in ((ga, 0), (gb, 1)):
                nc.gpsimd.dma_gather(
                    g_out[:], xt_t[:, :],
                    idxt[:, (G * 2 + br) * IDXW:(G * 2 + br + 1) * IDXW],
                    num_idxs=NIDX, num_idxs_reg=NIDX, elem_size=C,
                    transpose=True,
                )
            gx_grp[G] = (ga, gb)

        emit_gathers(0)
        emit_gathers(1)
        emit_gathers(2)

        pxs_sb = idxpool.tile([BPC, S2], i32, name="pxs_sb", tag="pxs_sb")
        nc.sync.dma_start(pxs_sb[:], pxs_t[:, :])
        pys_sb = idxpool.tile([BPC, S2], i32, name="pys_sb", tag="pys_sb")
        nc.sync.dma_start(pys_sb[:], pys_t[:, :])

        ones_row = wpool.tile([1, 128], bf16, name="ones_row", tag="ones_row")
        nc.vector.memset(ones_row[:], 1.0)

        # PE warm-up so the p-state ramps before real matmuls
        wmt = hps.tile([128, 128], f32, name="warm", tag="warm")
        for _ in range(48):
            nc.tensor.matmul(wmt[:], ones_row[:], ones_row[:],
                             start=True, stop=True)

        # ---------- weights ----------
        w1sb = wpool.tile([128, 2 * KC, HID], bf16, name="w1sb", tag="w1sb")
        nc.sync.dma_start(w1sb[:], w1_t[:, :, :])
        w2b = []
        for k in range(MJ):
            wb = wpool.tile([128, HID], bf16, name=f"w2b{k}", tag=f"w2b{k}")
            nc.sync.dma_start(wb[:], w2_t[k * 128:(k + 1) * 128, :])
            w2b.append(wb)
        w3b = []
        for k in range(MJ):
            wb = wpool.tile([128, OUT], bf16, name=f"w3b{k}", tag=f"w3b{k}")
            nc.sync.dma_start(wb[:], w3_t[k * 128:(k + 1) * 128, :])
            w3b.append(wb)
        b1c, b2c = [], []
        for j in range(MJ):
            t1 = wpool.tile([128, 1], f32, name=f"b1c{j}", tag=f"b1c{j}")
            nc.sync.dma_start(t1[:], b1_t[j * 128:(j + 1) * 128])
            b1c.append(t1)
            t2 = wpool.tile([128, 1], f32, name=f"b2c{j}", tag=f"b2c{j}")
            nc.sync.dma_start(t2[:], b2_t[j * 128:(j + 1) * 128])
            b2c.append(t2)
        b3c = wpool.tile([OUT, 1], f32, name="b3c", tag="b3c")
        nc.sync.dma_start(b3c[:], b3_t[:])

        # ---------- deltaxy on vector (gpsimd stays free for gathers) -----
        dsub = idxpool.tile([BPC, S2], i32, name="dsub", tag="dsub")
        nc.vector.tensor_tensor(dsub[:], pxs_sb[:], pys_sb[:], ALU.subtract)
        ddel = idxpool.tile([BPC, S2], f32, name="ddel", tag="ddel")
        nc.vector.tensor_scalar(ddel[:], dsub[:], float(H - 1), None,
                                op0=ALU.add)
        nc.sync.dma_start(bass.AP(delta_t, 0, [[S2, BPC], [1, S2]]), ddel[:])

        pred_all = opool.tile([OUT, BPC * S], f32, name="pred_all",
                              tag="pred_all")

        # ---------- per-pair compute ----------
        h1_pair = {}     # P -> [128, MJ, S2] bf16

        def emit_proj(P):
            G, q = divmod(P, 2)
            ga, gb = gx_grp[G]
            h1 = h1pool.tile([128, MJ, S2], bf16, name=f"h1_{P}", tag="h1")
            h1_pair[P] = h1
            zts = [zps.tile([128, S2], f32, name=f"zt{j}_{P}", tag=f"zt{j}")
                   for j in range(MJ)]
            # all branch-a matmuls first across j, so the first pair can
            # start as soon as the a-gather lands (b still in flight)
            for half, gx in ((0, ga), (1, gb)):
                for j in range(MJ):
                    for k in range(KC):
                        nc.tensor.matmul(
                            zts[j][:],
                            w1sb[:, half * KC + k, j * 128:(j + 1) * 128],
                            gx[:, k, q * S2:(q + 1) * S2],
                            start=(half == 0 and k == 0),
                            stop=(half == 1 and k == KC - 1),
                        )
                    if half == 1:
                        if j < 2:
                            nc.scalar.activation(h1[:, j, :], zts[j][:],
                                                 AF.Relu, bias=b1c[j][:])
                        else:
                            nc.vector.tensor_scalar(h1[:, j, :], zts[j][:],
                                                    b1c[j][:], 0.0,
                                                    op0=ALU.add, op1=ALU.max)

        def emit_tail(P):
            h1 = h1_pair.pop(P)
            h2 = h2pool.tile([128, MJ, S2], bf16, name=f"h2_{P}", tag="h2")
            for j in range(MJ):
                hp = hps.tile([128, S2], f32, name=f"h2ps{j}_{P}",
                              tag=f"hps{j % 2}")
                for k in range(MJ):
                    nc.tensor.matmul(
                        hp[:],
                        w2b[k][:, j * 128:(j + 1) * 128],
                        h1[:, k, :],
                        start=(k == 0), stop=(k == MJ - 1),
                    )
                if j < 2:
                    nc.scalar.activation(h2[:, j, :], hp[:], AF.Relu,
                                         bias=b2c[j][:])
                else:
                    nc.vector.tensor_scalar(h2[:, j, :], hp[:], b2c[j][:],
                                            0.0, op0=ALU.add, op1=ALU.max)
            pp = hps.tile([OUT, S2], f32, name=f"pps_{P}", tag="pps")
            for k in range(MJ):
                nc.tensor.matmul(pp[:], w3b[k][:], h2[:, k, :],
                                 start=(k == 0), stop=(k == MJ - 1))
            nc.vector.tensor_scalar(
                pred_all[:, P * S2:(P + 1) * S2], pp[:], b3c[:], None,
                op0=ALU.add,
            )
            if P % 2 == 1:
                qd = P // 2
                nc.sync.dma_start(
                    pred_t[:, qd * 2 * S2:(qd + 1) * 2 * S2],
                    pred_all[:, qd * 2 * S2:(qd + 1) * 2 * S2],
                )

        # ---------- main loop ----------
        for P in range(PAIRS):
            if P % 2 == 0:
                emit_gathers(P // 2 + 3)
            if P % 2 == 1:
                gx_grp.pop(P // 2 - 1, None)
            emit_proj(P)
            if P >= 1:
                emit_tail(P - 1)
        emit_tail(PAIRS - 1)

    nc.finalize()
    return nc


_NC = None


def _get_nc():
    global _NC
    if _NC is None:
        _install_ntff_hook()
        _NC = _build_nc()
    return _NC


def _make_in_maps(inputs):
    import ml_dtypes
    bf16 = ml_dtypes.bfloat16

    x = np.asarray(inputs["x"], dtype=np.float32).reshape(B, C, H * W_IMG)
    x = np.asarray(x, dtype=bf16)

    W1 = np.asarray(np.asarray(inputs["W1"], dtype=np.float32), dtype=bf16)
    # [2C, HID] -> [128, 12, HID]
    w1p = np.ascontiguousarray(
        W1.reshape(2, KC, 128, HID).transpose(2, 0, 1, 3)
    ).reshape(128, 2 * KC, HID)

    W2 = np.asarray(np.asarray(inputs["W2"], dtype=np.float32), dtype=bf16)
    W3 = np.asarray(np.asarray(inputs["W3"], dtype=np.float32), dtype=bf16)
    b1 = np.asarray(inputs["b1"], dtype=np.float32)
    b2 = np.asarray(inputs["b2"], dtype=np.float32)
    b3 = np.asarray(inputs["b3"], dtype=np.float32)
    pxs = np.asarray(inputs["pxs"], dtype=np.int32)
    pys = np.asarray(inputs["pys"], dtype=np.int32)
    idx_x = pxs[:, :, 0] * W_IMG + pxs[:, :, 1]     # [B, S]
    idx_y = pys[:, :, 0] * W_IMG + pys[:, :, 1]

    in_maps = []
    for c in range(N_CORES):
        sl = slice(c * BPC, (c + 1) * BPC)
        # xT [BPC*S, C]
        xT = np.ascontiguousarray(
            x[sl].transpose(0, 2, 1).reshape(BPC * S, C))

        # global row indices per (group, branch), wrapped for dge
        base = (np.arange(BPC, dtype=np.int32) * S)[:, None]   # [BPC, 1]
        gidx = np.zeros((NGRP, 2, NIDX), dtype=np.int16)
        for br, idx in ((0, idx_x[sl]), (1, idx_y[sl])):
            gl = (idx + base).astype(np.int16).reshape(NGRP, GB * S)
            gidx[:, br, :GB * S] = gl
        idxw = gidx.reshape(NGRP, 2, IDXW, 16).transpose(3, 0, 1, 2)
        idxw = np.tile(idxw.reshape(1, 16, NGRP, 2, IDXW), (8, 1, 1, 1, 1))
        idxw = np.ascontiguousarray(idxw).reshape(128, NGRP * 2 * IDXW)

        in_maps.append({
            "xT": xT,
            "W1": w1p, "W2": W2, "W3": W3,
            "b1": b1, "b2": b2, "b3": b3,
            "idxg": idxw,
            "pxs": np.ascontiguousarray(pxs[sl].reshape(BPC, S2)),
            "pys": np.ascontiguousarray(pys[sl].reshape(BPC, S2)),
        })
    return in_maps


def _run(inputs, trace=False):
    from concourse.bass_utils import run_bass_kernel_spmd

    nc = _get_nc()
    in_maps = _make_in_maps(inputs)
    res = run_bass_kernel_spmd(
        nc, in_maps, core_ids=list(range(N_CORES)), trace=trace
    )
    pred = np.concatenate(
        [np.ascontiguousarray(res.results[c]["predT"].T) for c in range(N_CORES)],
        axis=0,
    )
    delta = np.concatenate(
        [res.results[c]["deltaxy"] for c in range(N_CORES)], axis=0
    )
    return (pred, delta), res


def kernel(**inputs):
    (pred, delta), _ = _run(inputs, trace=False)
    return pred, delta


# revision 26
# speedup vs baseline: 2.4851x; 1.0014x over previous
"""Trainium2 Bass kernel for nn_DenseRelativeLoc.

Data-parallel over batch: 32 images per NeuronCore x 8 cores.

v3 gather-first design:
  * Host lays out x transposed as xT [BPC*196 rows, 768 ch] in DRAM.
  * dma_gather (DMA-engine indexed gather, gpsimd-triggered SWDGE) pulls
    the sampled rows straight from DRAM and transposes them into
    [128 ch, 6 ch-chunk, samples] SBUF tiles -- one gather per
    (4-batch group, branch). No one-hot matmuls, no z materialization.
  * Projection h1T[hid, s] = W1^T @ featsT runs on gathered features
    only; both branches accumulate into the same PSUM tile, so
    h1 = relu(psum + b1) comes out of a single activation.
  * GEMM2 / GEMM3 in bf16 as before.
"""
import sys
import types

import numpy as np

B, C, H, W_IMG = 256, 768, 14, 14
S = 196          # sample count == H*W
HID = 512
OUT = 2
N_CORES = 8
BPC = B // N_CORES      # 32 batches per core
PAIRS = BPC // 2        # 16
KC = C // 128           # 6 chunks per branch
MJ = HID // 128         # 4 HID chunks
S2 = 2 * S              # 392: pair width
GB = 4                  # batches per gather group
NGRP = BPC // GB        # 8 groups
NIDX = 896              # padded idx count per gather (GB*S=784 -> %128)
IDXW = NIDX // 16       # 56


def _install_ntff_hook():
    try:
        import antenv.axon_hooks  # noqa: F401
        return
    except ImportError:
        pass
    try:
        from trn_agent_boot.trn_boot import _ntff_profile_via_ctypes
        hook = _ntff_profile_via_ctypes("/opt/axon/libaxon_pjrt.so")
    except Exception:
        hook = None
    mod = types.ModuleType("antenv.axon_hooks")
    mod.get_axon_ntff_profile_hook = lambda: hook
    sys.modules["antenv.axon_hooks"] = mod


def _build_nc():
    from contextlib import ExitStack

    import concourse.bass as bass
    import concourse.bacc as bacc
    import concourse.mybir as mybir
    import concourse.tile as tile

    dt = mybir.dt
    f32, bf16, i32, i16 = dt.float32, dt.bfloat16, dt.int32, dt.int16
    AF = mybir.ActivationFunctionType
    ALU = mybir.AluOpType

    nc = bacc.Bacc(None, target_bir_lowering=False)

    xt_t = nc.dram_tensor("xT", [BPC * S, C], bf16, kind="ExternalInput")
    w1_t = nc.dram_tensor("W1", [128, 2 * KC, HID], bf16, kind="ExternalInput")
    w2_t = nc.dram_tensor("W2", [HID, HID], bf16, kind="ExternalInput")
    w3_t = nc.dram_tensor("W3", [HID, OUT], bf16, kind="ExternalInput")
    b1_t = nc.dram_tensor("b1", [HID], f32, kind="ExternalInput")
    b2_t = nc.dram_tensor("b2", [HID], f32, kind="ExternalInput")
    b3_t = nc.dram_tensor("b3", [OUT], f32, kind="ExternalInput")
    idx_t = nc.dram_tensor("idxg", [128, NGRP * 2 * IDXW], i16,
                           kind="ExternalInput")
    pxs_t = nc.dram_tensor("pxs", [BPC, S2], i32, kind="ExternalInput")
    pys_t = nc.dram_tensor("pys", [BPC, S2], i32, kind="ExternalInput")
    pred_t = nc.dram_tensor("predT", [OUT, BPC * S], f32, kind="ExternalOutput")
    delta_t = nc.dram_tensor("deltaxy", [BPC * S, OUT], f32,
                             kind="ExternalOutput")

    with ExitStack() as ctx:
        tc = ctx.enter_context(tile.TileContext(nc))
        wpool = ctx.enter_context(tc.tile_pool(name="w", bufs=1))
        gxpool = ctx.enter_context(tc.tile_pool(name="gx", bufs=3))
        h1pool = ctx.enter_context(tc.tile_pool(name="h1", bufs=2))
        h2pool = ctx.enter_context(tc.tile_pool(name="h2", bufs=2))
        opool = ctx.enter_context(tc.tile_pool(name="op", bufs=1))
        idxpool = ctx.enter_context(tc.tile_pool(name="idx", bufs=1))
        zps = ctx.enter_context(tc.tile_pool(name="zps", bufs=1, space="PSUM"))
        hps = ctx.enter_context(tc.tile_pool(name="hps", bufs=1, space="PSUM"))

        # ---------- small tensors ----------
        idxt = idxpool.tile([128, NGRP * 2 * IDXW], i16, name="idxt",
                            tag="idxt")
        nc.sync.dma_start(idxt[:], idx_t[:, :])

        # ---------- gathers first: keep the gpsimd/SWDGE queue clear ------
        gx_grp = {}      # G -> (gxa, gxb) [128, KC, NIDX] bf16

        def emit_gathers(G):
            if G in gx_grp or G >= NGRP:
                return
            ga = gxpool.tile([128, KC, NIDX], bf16, name=f"gxa{G}", tag="gxa")
            gb = gxpool.tile([128, KC, NIDX], bf16, name=f"gxb{G}", tag="gxb")
            for g_out, br in ((ga, 0), (gb, 1)):
                nc.gpsimd.dma_gather(
                    g_out[:], xt_t[:, :],
                    idxt[:, (G * 2 + br) * IDXW:(G * 2 + br + 1) * IDXW],
                    num_idxs=NIDX, num_idxs_reg=NIDX, elem_size=C,
                    transpose=True,
                )
            gx_grp[G] = (ga, gb)

        emit_gathers(0)
        emit_gathers(1)
        emit_gathers(2)

        pxs_sb = idxpool.tile([BPC, S2], i32, name="pxs_sb", tag="pxs_sb")
        nc.sync.dma_start(pxs_sb[:], pxs_t[:, :])
        pys_sb = idxpool.tile([BPC, S2], i32, name="pys_sb", tag="pys_sb")
        nc.sync.dma_start(pys_sb[:], pys_t[:, :])

        ones_row = wpool.tile([1, 128], bf16, name="ones_row", tag="ones_row")
        nc.vector.memset(ones_row[:], 1.0)

        # PE warm-up so the p-state ramps before real matmuls
        wmt = hps.tile([128, 128], f32, name="warm", tag="warm")
        for _ in range(48):
            nc.tensor.matmul(wmt[:], ones_row[:], ones_row[:],
                             start=True, stop=True)

        # ---------- weights ----------
        w1sb = wpool.tile([128, 2 * KC, HID], bf16, name="w1sb", tag="w1sb")
        nc.sync.dma_start(w1sb[:], w1_t[:, :, :])
        w2b = []
        for k in range(MJ):
            wb = wpool.tile([128, HID], bf16, name=f"w2b{k}", tag=f"w2b{k}")
            nc.sync.dma_start(wb[:], w2_t[k * 128:(k + 1) * 128, :])
            w2b.append(wb)
        w3b = []
        for k in range(MJ):
            wb = wpool.tile([128, OUT], bf16, name=f"w3b{k}", tag=f"w3b{k}")
            nc.sync.dma_start(wb[:], w3_t[k * 128:(k + 1) * 128, :])
            w3b.append(wb)
        b1c, b2c = [], []
        for j in range(MJ):
            t1 = wpool.tile([128, 1], f32, name=f"b1c{j}", tag=f"b1c{j}")
            nc.sync.dma_start(t1[:], b1_t[j * 128:(j + 1) * 128])
            b1c.append(t1)
            t2 = wpool.tile([128, 1], f32, name=f"b2c{j}", tag=f"b2c{j}")
            nc.sync.dma_start(t2[:], b2_t[j * 128:(j + 1) * 128])
            b2c.append(t2)
        b3c = wpool.tile([OUT, 1], f32, name="b3c", tag="b3c")
        nc.sync.dma_start(b3c[:], b3_t[:])

        # ---------- deltaxy on vector (gpsimd stays free for gathers) -----
        dsub = idxpool.tile([BPC, S2], i32, name="dsub", tag="dsub")
        nc.vector.tensor_tensor(dsub[:], pxs_sb[:], pys_sb[:], ALU.subtract)
        ddel = idxpool.tile([BPC, S2], f32, name="ddel", tag="ddel")
        nc.vector.tensor_scalar(ddel[:], dsub[:], float(H - 1), None,
                                op0=ALU.add)
        nc.sync.dma_start(bass.AP(delta_t, 0, [[S2, BPC], [1, S2]]), ddel[:])

        pred_all = opool.tile([OUT, BPC * S], f32, name="pred_all",
                              tag="pred_all")

        # ---------- per-pair compute ----------
        h1_pair = {}     # P -> [128, MJ, S2] bf16

        def emit_proj(P):
            G, q = divmod(P, 2)
            ga, gb = gx_grp[G]
            h1 = h1pool.tile([128, MJ, S2], bf16, name=f"h1_{P}", tag="h1")
            h1_pair[P] = h1
            zts = [zps.tile([128, S2], f32, name=f"zt{j}_{P}", tag=f"zt{j}")
                   for j in range(MJ)]
            # all branch-a matmuls first across j, so the first pair can
            # start as soon as the a-gather lands (b still in flight)
            for half, gx in ((0, ga), (1, gb)):
                for j in range(MJ):
                    for k in range(KC):
                        nc.tensor.matmul(
                            zts[j][:],
                            w1sb[:, half * KC + k, j * 128:(j + 1) * 128],
                            gx[:, k, q * S2:(q + 1) * S2],
                            start=(half == 0 and k == 0),
                            stop=(half == 1 and k == KC - 1),
                        )
                    if half == 1:
                        if j < 2:
                            nc.scalar.activation(h1[:, j, :], zts[j][:],
                                                 AF.Relu, bias=b1c[j][:])
                        else:
                            nc.vector.tensor_scalar(h1[:, j, :], zts[j][:],
                                                    b1c[j][:], 0.0,
                                                    op0=ALU.add, op1=ALU.max)

        def emit_tail(P):
            h1 = h1_pair.pop(P)
            h2 = h2pool.tile([128, MJ, S2], bf16, name=f"h2_{P}", tag="h2")
            for j in range(MJ):
                hp = hps.tile([128, S2], f32, name=f"h2ps{j}_{P}",
                              tag=f"hps{j % 2}")
                for k in range(MJ):
                    nc.tensor.matmul(
                        hp[:],
                        w2b[k][:, j * 128:(j + 1) * 128],
                        h1[:, k, :],
                        start=(k == 0), stop=(k == MJ - 1),
                    )
                if j < 2:
                    nc.scalar.activation(h2[:, j, :], hp[:], AF.Relu,
                                         bias=b2c[j][:])
                else:
                    nc.vector.tensor_scalar(h2[:, j, :], hp[:], b2c[j][:],
                                            0.0, op0=ALU.add, op1=ALU.max)
            pp = hps.tile([OUT, S2], f32, name=f"pps_{P}", tag="pps")
            for k in range(MJ):
                nc.tensor.matmul(pp[:], w3b[k][:], h2[:, k, :],
                                 start=(k == 0), stop=(k == MJ - 1))
            nc.vector.tensor_scalar(
                pred_all[:, P * S2:(P + 1) * S2], pp[:], b3c[:], None,
                op0=ALU.add,
            )
            if P % 2 == 1:
                qd = P // 2
                nc.sync.dma_start(
                    pred_t[:, qd * 2 * S2:(qd + 1) * 2 * S2],
                    pred_all[:, qd * 2 * S2:(qd + 1) * 2 * S2],
                )

        # ---------- main loop ----------
        for P in range(PAIRS):
            if P % 2 == 0:
                emit_gathers(P // 2 + 3)
            if P % 2 == 1:
                gx_grp.pop(P // 2 - 1, None)
            emit_proj(P)
            if P >= 1:
                emit_tail(P - 1)
        emit_tail(PAIRS - 1)

    nc.finalize()
    return nc


_NC = None


def _get_nc():
    global _NC
    if _NC is None:
        _install_ntff_hook()
        _NC = _build_nc()
    return _NC


def _make_in_maps(inputs):
    import ml_dtypes
    bf16 = ml_dtypes.bfloat16

    x = np.asarray(inputs["x"], dtype=np.float32).reshape(B, C, H * W_IMG)
    x = np.asarray(x, dtype=bf16)

    W1 = np.asarray(np.asarray(inputs["W1"], dtype=np.float32), dtype=bf16)
    # [2C, HID] -> [128, 12, HID]
    w1p = np.ascontiguousarray(
        W1.reshape(2, KC, 128, HID).transpose(2, 0, 1, 3)
    ).reshape(128, 2 * KC, HID)

    W2 = np.asarray(np.asarray(inputs["W2"], dtype=np.float32), dtype=bf16)
    W3 = np.asarray(np.asarray(inputs["W3"], dtype=np.float32), dtype=bf16)
    b1 = np.asarray(inputs["b1"], dtype=np.float32)
    b2 = np.asarray(inputs["b2"], dtype=np.float32)
    b3 = np.asarray(inputs["b3"], dtype=np.float32)
    pxs = np.asarray(inputs["pxs"], dtype=np.int32)
    pys = np.asarray(inputs["pys"], dtype=np.int32)
    idx_x = pxs[:, :, 0] * W_IMG + pxs[:, :, 1]     # [B, S]
    idx_y = pys[:, :, 0] * W_IMG + pys[:, :, 1]

    in_maps = []
    for c in range(N_CORES):
        sl = slice(c * BPC, (c + 1) * BPC)
        # xT [BPC*S, C]
        xT = np.ascontiguousarray(
            x[sl].transpose(0, 2, 1).reshape(BPC * S, C))

        # global row indices per (group, branch), wrapped for dge
        base = (np.arange(BPC, dtype=np.int32) * S)[:, None]   # [BPC, 1]
        gidx = np.zeros((NGRP, 2, NIDX), dtype=np.int16)
        for br, idx in ((0, idx_x[sl]), (1, idx_y[sl])):
            gl = (idx + base).astype(np.int16).reshape(NGRP, GB * S)
            gidx[:, br, :GB * S] = gl
        idxw = gidx.reshape(NGRP, 2, IDXW, 16).transpose(3, 0, 1, 2)
        idxw = np.tile(idxw.reshape(1, 16, NGRP, 2, IDXW), (8, 1, 1, 1, 1))
        idxw = np.ascontiguousarray(idxw).reshape(128, NGRP * 2 * IDXW)

        in_maps.append({
            "xT": xT,
            "W1": w1p, "W2": W2, "W3": W3,
            "b1": b1, "b2": b2, "b3": b3,
            "idxg": idxw,
            "pxs": np.ascontiguousarray(pxs[sl].reshape(BPC, S2)),
            "pys": np.ascontiguousarray(pys[sl].reshape(BPC, S2)),
        })
    return in_maps


def _run(inputs, trace=False):
    from concourse.bass_utils import run_bass_kernel_spmd

    nc = _get_nc()
    in_maps = _make_in_maps(inputs)
    res = run_bass_kernel_spmd(
        nc, in_maps, core_ids=list(range(N_CORES)), trace=trace
    )
    pred = np.concatenate(
        [np.ascontiguousarray(res.results[c]["predT"].T) for c in range(N_CORES)],
        axis=0,
    )
    delta = np.concatenate(
        [res.results[c]["deltaxy"] for c in range(N_CORES)], axis=0
    )
    return (pred, delta), res


def kernel(**inputs):
    (pred, delta), _ = _run(inputs, trace=False)
    return pred, delta
